# revision 4
# baseline (speedup 1.0000x reference)
"""AttentionEncoder TRN2 Bass kernel.

Data-parallel over batch: B=8 samples -> 8 NeuronCores, one sample per core.
All matmuls run in float32r (HW rounds fp32 to 11 mantissa bits, RNE); inputs
that feed matmuls are pre-rounded on host so plain (non-casting) DMAs work.

Per-core math (S=2048, D=1024, K=64), all layouts chosen so no on-chip
transposes are needed except res -> resT before the FF matmul:

  phase 0: values = x@Wv (s-major, stationary xT), kqT = [Wk*s|Wq]^T@x^T,
           biasesT = Wb^T@x^T  (tile_k rows0-63 = scaled keysT, row64 = biasesT;
           tile_q rows0-63 = queriesT, row64 = ones)
  phase 1: per 512-col superblock: scoresT[j,i] = sigmoid(qk + bias) via a
           single 65-row contraction; attention accumulated s-major;
           LN1 via bn_stats; res spilled to DRAM (f32r)
  phase 2: per 128-row chunk: PE-transpose res, FF matmul, relu+residual in
           one DVE op, LN2, DMA out.
"""
import numpy as np
from contextlib import ExitStack

import concourse.bass as bass
import concourse.tile as tile
from concourse import bacc, mybir
from concourse.bass_utils import run_bass_kernel_spmd
from concourse.alu_op_type import AluOpType

F32 = mybir.dt.float32
F32R = mybir.dt.float32r
ACTF = mybir.ActivationFunctionType

B, S, D, K = 8, 2048, 1024, 64
EPS = 1e-5
NCORES = 8
SB = 512          # superblock width (scores free dim)
NSB = S // SB     # 4
NC = S // 128     # 16 s-chunks
ND2 = D // 512    # 2 d-tiles


def _rne11(a: np.ndarray) -> np.ndarray:
    """Round fp32 to fp32r (RNE to 11 mantissa bits), bit-exact vs HW DMA cast."""
    bits = np.ascontiguousarray(a, dtype=np.float32).view(np.uint32)
    drop = 12
    half = np.uint32(1 << (drop - 1))
    lsb = (bits >> drop) & np.uint32(1)
    rounded = bits + (half - np.uint32(1)) + lsb
    out = ((rounded >> drop) << drop).view(np.float32)
    return out.copy()


def build_program(flags):
    have_bkq, have_bb, have_bv, have_b1, have_gb = flags
    nc = bacc.Bacc(trn_type="TRN2")

    xt_d = nc.declare_dram_parameter("xt", [D, S], F32R, isOutput=False)
    x_d = nc.declare_dram_parameter("x", [S, D], F32, isOutput=False)
    wkq_d = nc.declare_dram_parameter("wkq", [D, 128], F32R, isOutput=False)
    wb_d = nc.declare_dram_parameter("wb", [D, 1], F32R, isOutput=False)
    wv_d = nc.declare_dram_parameter("wv", [D, D], F32R, isOutput=False)
    w1_d = nc.declare_dram_parameter("w1", [D, D], F32R, isOutput=False)
    ones_d = nc.declare_dram_parameter("onesrow", [1, S], F32R, isOutput=False)
    iden_d = nc.declare_dram_parameter("iden", [128, 128], F32R, isOutput=False)
    g1_d = nc.declare_dram_parameter("g1", [1, D], F32, isOutput=False)
    be1_d = nc.declare_dram_parameter("be1", [1, D], F32, isOutput=False)
    bkq_d = nc.declare_dram_parameter("bkq", [1, 128], F32R, isOutput=False)
    bb_d = nc.declare_dram_parameter("bb", [1, 1], F32R, isOutput=False)
    bv_d = nc.declare_dram_parameter("bv", [1, D], F32R, isOutput=False)
    b1_d = nc.declare_dram_parameter("b1", [1, D], F32R, isOutput=False)
    out_d = nc.declare_dram_parameter("out", [S, D], F32, isOutput=True)

    res_spill = nc.dram_tensor("res_spill", [NC, 128, D], F32R)

    with tile.TileContext(nc) as tc, ExitStack() as top:
        const = top.enter_context(tc.tile_pool(name="const", bufs=1))
        kqp = top.enter_context(tc.tile_pool(name="kqp", bufs=1))
        vp = top.enter_context(tc.tile_pool(name="vp", bufs=1))

        # ---- constants
        eps_t = const.tile([128, 1], F32)
        nc.vector.memset(eps_t, EPS)
        zero_t = const.tile([128, 1], F32)
        nc.vector.memset(zero_t, 0.0)
        ones_t = const.tile([1, S], F32R)
        nc.sync.dma_start(ones_t, ones_d.ap())
        if have_gb:
            g1_b = const.tile([128, D], F32)
            nc.sync.dma_start(g1_b, g1_d.ap().partition_broadcast(128))
            be1_b = const.tile([128, D], F32)
            nc.sync.dma_start(be1_b, be1_d.ap().partition_broadcast(128))
        if have_bkq:
            bkq_t = const.tile([1, 128], F32R)
            nc.sync.dma_start(bkq_t, bkq_d.ap())
        if have_bb:
            bb_t = const.tile([1, 1], F32R)
            nc.sync.dma_start(bb_t, bb_d.ap())
        if have_bv:
            bv_t = const.tile([1, D], F32R)
            nc.sync.dma_start(bv_t, bv_d.ap())
        if have_b1:
            b1_t = const.tile([1, D], F32R)
            nc.sync.dma_start(b1_t, b1_d.ap())

        # ---- kq/bias output tiles (rows 0..64)
        tile_k = kqp.tile([65, S], F32R)   # rows0-63 scaled keysT, row64 biasesT
        tile_q = kqp.tile([65, S], F32R)   # rows0-63 queriesT, row64 ones
        nc.sync.dma_start(tile_q[64:65, :], ones_d.ap())

        # ---- v_sb: values s-major, resident through phase 1
        v_sb = vp.tile([128, NC, D], F32R)

        # ================= phase 0: projections =================
        with ExitStack() as ph0:
            xp = ph0.enter_context(tc.tile_pool(name="xp", bufs=1))
            wp = ph0.enter_context(tc.tile_pool(name="wp", bufs=1))
            pv = ph0.enter_context(tc.tile_pool(name="pv", bufs=4, space="PSUM"))
            pkq = ph0.enter_context(tc.tile_pool(name="pkq", bufs=2, space="PSUM"))
            pb = ph0.enter_context(tc.tile_pool(name="pb", bufs=2, space="PSUM"))

            xt_t = xp.tile([128, 8, S], F32R)
            nc.sync.dma_start(xt_t, xt_d.rearrange("(c p) s -> p c s", p=128))
            wv_t = wp.tile([128, 8, D], F32R, tag="wv")
            nc.sync.dma_start(wv_t, wv_d.rearrange("(c p) d -> p c d", p=128))
            wkq_t = xp.tile([128, 8, 128], F32R)
            nc.sync.dma_start(wkq_t, wkq_d.rearrange("(c p) m -> p c m", p=128))
            wb_t = xp.tile([128, 8, 1], F32R)
            nc.sync.dma_start(wb_t, wb_d.rearrange("(c p) o -> p c o", p=128))

            # values: out[s-chunk,128 x d-512], stationary xT slices
            for sc in range(NC):
                for dt2 in range(ND2):
                    pvt = pv.tile([128, 512], F32, tag="v")
                    nmm = 8 + (1 if have_bv else 0)
                    for k in range(8):
                        nc.tensor.matmul(
                            pvt, xt_t[:, k, sc * 128:(sc + 1) * 128],
                            wv_t[:, k, dt2 * 512:(dt2 + 1) * 512],
                            start=(k == 0), stop=(k == nmm - 1))
                    if have_bv:
                        nc.tensor.matmul(
                            pvt, ones_t[:, 0:128],
                            bv_t[:, dt2 * 512:(dt2 + 1) * 512],
                            start=False, stop=True)
                    nc.scalar.activation(
                        v_sb[:, sc, dt2 * 512:(dt2 + 1) * 512], pvt, ACTF.Copy)

            # kq + biases per s-tile of 512
            for st in range(NSB):
                sl = slice(st * SB, (st + 1) * SB)
                pk = pkq.tile([128, 512], F32, tag="kq")
                nmm = 8 + (1 if have_bkq else 0)
                for k in range(8):
                    nc.tensor.matmul(pk, wkq_t[:, k, :], xt_t[:, k, sl],
                                     start=(k == 0), stop=(k == nmm - 1))
                if have_bkq:
                    nc.tensor.matmul(pk, bkq_t, ones_t[:, sl],
                                     start=False, stop=True)
                nc.scalar.activation(tile_k[0:64, sl], pk[0:64, :], ACTF.Copy)
                nc.scalar.activation(tile_q[0:64, sl], pk[64:128, :], ACTF.Copy)

                pbt = pb.tile([1, 512], F32, tag="b")
                nmm = 8 + (1 if have_bb else 0)
                for k in range(8):
                    nc.tensor.matmul(pbt, wb_t[:, k, :], xt_t[:, k, sl],
                                     start=(k == 0), stop=(k == nmm - 1))
                if have_bb:
                    nc.tensor.matmul(pbt, bb_t, ones_t[:, sl],
                                     start=False, stop=True)
                nc.scalar.activation(tile_k[64:65, sl], pbt, ACTF.Copy)

        # ================= phase 1: scores + attention + LN1 =================
        w1p = top.enter_context(tc.tile_pool(name="w1p", bufs=1))
        w1_t = w1p.tile([128, 8, D], F32R)
        nc.sync.dma_start(w1_t, w1_d.rearrange("(c p) d -> p c d", p=128))

        if True:
            with ExitStack() as ph1i:
                strips = ph1i.enter_context(tc.tile_pool(name="strips", bufs=1))
                xin = ph1i.enter_context(tc.tile_pool(name="xin", bufs=3))
                rpre = ph1i.enter_context(tc.tile_pool(name="rpre", bufs=3))
                rout = ph1i.enter_context(tc.tile_pool(name="rout", bufs=3))
                stat = ph1i.enter_context(tc.tile_pool(name="stat", bufs=6))
                ps_s = ph1i.enter_context(
                    tc.tile_pool(name="ps_s", bufs=3, space="PSUM"))
                ps_a = ph1i.enter_context(
                    tc.tile_pool(name="ps_a", bufs=4, space="PSUM"))

                for sb in range(NSB):
                    isl = slice(sb * SB, (sb + 1) * SB)
                    strip = strips.tile([128, NC, SB], F32R, tag="strip")
                    for j in range(NC):
                        pst = ps_s.tile([128, SB], F32, tag="s")
                        nc.tensor.matmul(
                            pst, tile_q[:, j * 128:(j + 1) * 128],
                            tile_k[:, isl], start=True, stop=True)
                        nc.scalar.activation(strip[:, j, :], pst, ACTF.Sigmoid)

                    for l in range(4):
                        c = sb * 4 + l   # global 128-row chunk
                        pa = [ps_a.tile([128, 512], F32, tag="a", name=f"pa{_d}")
                              for _d in range(ND2)]
                        for dt2 in range(ND2):
                            for j in range(NC):
                                nc.tensor.matmul(
                                    pa[dt2],
                                    strip[:, j, l * 128:(l + 1) * 128],
                                    v_sb[:, j, dt2 * 512:(dt2 + 1) * 512],
                                    start=(j == 0), stop=(j == NC - 1))
                        # LN1
                        x_t = xin.tile([128, D], F32, tag="x")
                        nc.sync.dma_start(x_t, x_d[c * 128:(c + 1) * 128, :])
                        rp = rpre.tile([128, D], F32, tag="rp")
                        for dt2 in range(ND2):
                            dsl = slice(dt2 * 512, (dt2 + 1) * 512)
                            nc.vector.tensor_tensor(
                                rp[:, dsl], pa[dt2], x_t[:, dsl],
                                op=AluOpType.add)
                        st_t = stat.tile([128, 2, 6], F32, tag="bst")
                        for g in range(2):
                            nc.vector.bn_stats(st_t[:, g, :],
                                               rp[:, g * 512:(g + 1) * 512])
                        mv = stat.tile([128, 2], F32, tag="mv")
                        nc.vector.bn_aggr(mv, st_t)
                        rstd = stat.tile([128, 1], F32, tag="rstd")
                        nc.scalar.activation(rstd, mv[:, 1:2], ACTF.Sqrt,
                                             bias=eps_t, scale=1.0)
                        nc.vector.reciprocal(out=rstd, in_=rstd)
                        rt = rout.tile([128, D], F32R, tag="res")
                        if have_gb:
                            t1 = rpre.tile([128, D], F32, tag="t1")
                            nc.vector.scalar_tensor_tensor(
                                out=t1, in0=rp, scalar=mv[:, 0:1], in1=g1_b,
                                op0=AluOpType.subtract, op1=AluOpType.mult)
                            nc.vector.scalar_tensor_tensor(
                                out=rt, in0=t1, scalar=rstd, in1=be1_b,
                                op0=AluOpType.mult, op1=AluOpType.add)
                        else:
                            nc.vector.tensor_scalar(
                                out=rt, in0=rp, scalar1=mv[:, 0:1],
                                scalar2=rstd, op0=AluOpType.subtract,
                                op1=AluOpType.mult)
                        nc.sync.dma_start(res_spill[c], rt)

        # ================= phase 2: FF + LN2 =================
        with ExitStack() as ph2:
            rin = ph2.enter_context(tc.tile_pool(name="rin", bufs=3))
            rtp = ph2.enter_context(tc.tile_pool(name="rtp", bufs=3))
            f2 = ph2.enter_context(tc.tile_pool(name="f2", bufs=3))
            ostage = ph2.enter_context(tc.tile_pool(name="ostage", bufs=3))
            stat2 = ph2.enter_context(tc.tile_pool(name="stat2", bufs=6))
            ps_t = ph2.enter_context(
                tc.tile_pool(name="ps_t", bufs=2, space="PSUM"))
            ps_f = ph2.enter_context(
                tc.tile_pool(name="ps_f", bufs=4, space="PSUM"))

            res_t = [None] * NC
            rT = [None] * NC

            def stage_tr(c):
                r = rin.tile([128, D], F32R, tag="rin")
                nc.sync.dma_start(r, res_spill[c])
                ptr = ps_t.tile([128, D], F32R, tag="tr")
                for k in range(8):
                    ksl = slice(k * 128, (k + 1) * 128)
                    nc.tensor.transpose(ptr[:, ksl], r[:, ksl], iden_t)
                rt_ = rtp.tile([128, 8, 128], F32R, tag="rT")
                nc.scalar.activation(rt_, ptr, ACTF.Copy)
                res_t[c] = r
                rT[c] = rt_

            def stage_ff(c):
                pf = [ps_f.tile([128, 512], F32, tag="f", name=f"pf{_d}")
                      for _d in range(ND2)]
                nmm = 8 + (1 if have_b1 else 0)
                for dt2 in range(ND2):
                    dsl = slice(dt2 * 512, (dt2 + 1) * 512)
                    for k in range(8):
                        nc.tensor.matmul(pf[dt2], rT[c][:, k, :],
                                         w1_t[:, k, dsl],
                                         start=(k == 0), stop=(k == nmm - 1))
                    if have_b1:
                        nc.tensor.matmul(pf[dt2], ones_t[:, 0:128],
                                         b1_t[:, dsl], start=False, stop=True)
                r2 = f2.tile([128, D], F32, tag="r2")
                for dt2 in range(ND2):
                    dsl = slice(dt2 * 512, (dt2 + 1) * 512)
                    nc.vector.scalar_tensor_tensor(
                        out=r2[:, dsl], in0=pf[dt2], scalar=zero_t,
                        in1=res_t[c].bitcast(F32)[:, dsl],
                        op0=AluOpType.max, op1=AluOpType.add)
                st_t = stat2.tile([128, 2, 6], F32, tag="bst2")
                for g in range(2):
                    nc.vector.bn_stats(st_t[:, g, :],
                                       r2[:, g * 512:(g + 1) * 512])
                mv = stat2.tile([128, 2], F32, tag="mv2")
                nc.vector.bn_aggr(mv, st_t)
                rstd = stat2.tile([128, 1], F32, tag="rstd2")
                nc.scalar.activation(rstd, mv[:, 1:2], ACTF.Sqrt,
                                     bias=eps_t, scale=1.0)
                nc.vector.reciprocal(out=rstd, in_=rstd)
                o_t = ostage.tile([128, D], F32, tag="o")
                nc.vector.tensor_scalar(
                    out=o_t, in0=r2, scalar1=mv[:, 0:1], scalar2=rstd,
                    op0=AluOpType.subtract, op1=AluOpType.mult)
                nc.sync.dma_start(out_d[c * 128:(c + 1) * 128, :], o_t)

            iden_t = f2.tile([128, 128], F32R, tag="iden")
            nc.sync.dma_start(iden_t, iden_d.ap())

            for c in range(NC + 1):
                if c < NC:
                    stage_tr(c)
                if c >= 1:
                    stage_ff(c - 1)

    nc.finalize()
    return nc


_PROGRAM_CACHE = {}


def _get_program(flags):
    if flags not in _PROGRAM_CACHE:
        _PROGRAM_CACHE[flags] = build_program(flags)
    return _PROGRAM_CACHE[flags]


def kernel(x, Wk, bk, Wq, bq, Wv, bv, Wb, bb, W1, b1, g1, be1):
    import math
    scale = 1.0 / math.sqrt(K)
    flags = (
        bool(np.any(bk) or np.any(bq)),
        bool(np.any(bb)),
        bool(np.any(bv)),
        bool(np.any(b1)),
        bool(np.any(g1 != 1.0) or np.any(be1)),
    )
    nc = _get_program(flags)

    wkq = _rne11(np.concatenate([Wk * scale, Wq], axis=1))      # [D,128]
    wb = _rne11(Wb)                                             # [D,1]
    wv = _rne11(Wv)                                             # [D,D]
    w1 = _rne11(W1)                                             # [D,D]
    onesrow = np.ones((1, S), dtype=np.float32)
    iden = np.eye(128, dtype=np.float32)
    bkq = _rne11(np.concatenate([bk * scale, bq])[None, :])
    bbr = _rne11(np.asarray(bb, dtype=np.float32).reshape(1, 1))
    bvr = _rne11(np.asarray(bv, dtype=np.float32)[None, :])
    b1r = _rne11(np.asarray(b1, dtype=np.float32)[None, :])
    g1r = np.asarray(g1, dtype=np.float32)[None, :]
    be1r = np.asarray(be1, dtype=np.float32)[None, :]

    xt = _rne11(np.ascontiguousarray(np.transpose(x, (0, 2, 1))))  # [B,D,S]

    in_maps = []
    for b in range(B):
        in_maps.append(dict(
            xt=xt[b], x=np.ascontiguousarray(x[b]), wkq=wkq, wb=wb, wv=wv,
            w1=w1, onesrow=onesrow, iden=iden, g1=g1r, be1=be1r, bkq=bkq, bb=bbr,
            bv=bvr, b1=b1r))

    res = run_bass_kernel_spmd(nc, in_maps, list(range(NCORES)), trace=False)
    out = np.stack([res.results[b]["out"] for b in range(B)], axis=0)
    return out.astype(np.float32)
